# revision 1
# baseline (speedup 1.0000x reference)
"""CSGNN Trainium kernel v2: host preprocessing + Bass/Tile kernel builder.

Data-parallel over graphs: nodes partitioned at graph boundaries across 8
cores, edges live on their dst node's core grouped by 128-node dst blocks.

Structure (all heavy tensors bf16):
- Per layer t = h @ W computed per-core and AllGathered in TWO chunks
  (first/second half of each core's node blocks) so the collective overlaps
  with aggregation; gather tables are [8*3200, H] each, indices fit int16.
  Chunk collectives are issued mid-pass-B as soon as their transforms land.
- Aggregation runs in two passes (A-half tiles -> f32 accumulator in SBUF,
  then B-half tiles + self-loop + bias + accumulator add + relu), in waves
  of 4 blocks: one e read, one one-hot build (double-broadcast is_equal),
  and one g*e multiply per wave to amortize DVE/DMA fixed costs.
- norm is folded into e at edge-MLP time; the edge MLP computes e
  edge-major directly (lhsT = h1 tile slice), avoiding transposes.
- Gather queue_nums are rewritten post-compile to match the scheduler's
  DMASW sem-lane order (emission-order rotation races after reordering).
"""
import numpy as np
import ml_dtypes

import concourse.bacc as bacc
import concourse.bass as bass
import concourse.tile as tile
import concourse.mybir as mybir
from concourse import library_config

F32 = mybir.dt.float32
BF16 = mybir.dt.bfloat16
NC = 8
H = 128
P = 128
MAX_GATHER_IDX = 1024  # per dma_gather call (HW hangs at 2048)
CHUNK = 4  # edge-MLP tiles per chunk (512 edges)


def _ceil(a, b):
    return -(-a // b)


def _bf(x):
    return np.asarray(x).astype(ml_dtypes.bfloat16)


def preprocess(x, edge_attr, edge_index, batch, n_graphs):
    """Compute the sharding plan + per-core host arrays."""
    x = np.asarray(x, np.float32)
    edge_attr = np.asarray(edge_attr, np.float32)
    ei = np.asarray(edge_index, np.int64)
    batch = np.asarray(batch, np.int64)
    N, F = x.shape
    E, Fe = edge_attr.shape
    G = n_graphs
    assert G % NC == 0
    gpc = G // NC  # graphs per core

    src, dst = ei[0], ei[1]

    # node partition at graph boundaries
    node_start = np.searchsorted(batch, np.arange(0, G + 1, gpc), side="left")
    n_local = np.diff(node_start)
    n_pad = max(_ceil(int(n_local.max()), 2 * P) * 2 * P, 2 * P)
    NB = n_pad // P
    n_half = n_pad // 2
    NBH = NB // 2  # blocks per half
    assert NC * n_half < 32768  # int16 gather indices per chunk table

    # degrees / normalization (index-derived scalars)
    deg = 1.0 + np.bincount(dst, minlength=N).astype(np.float32)
    dis = 1.0 / np.sqrt(deg)
    norm = (dis[src] * dis[dst]).astype(np.float32)
    dis2 = (dis * dis).astype(np.float32)

    core_of = np.searchsorted(node_start[1:], np.arange(N), side="right")
    local_of = np.arange(N) - node_start[core_of]
    half_of = (local_of >= n_half).astype(np.int64)
    # row in the half-chunk gather table
    tidx = core_of * n_half + (local_of - half_of * n_half)

    gsrc_half = half_of[src]
    gsrc_tidx = tidx[src]
    dst_core = core_of[dst]
    dst_local = local_of[dst]
    blk = dst_local // P
    dst_in_blk = dst_local % P

    # bucket (core, block, half-of-src)
    key = (dst_core * NB + blk) * 2 + gsrc_half
    order = np.argsort(key, kind="stable")
    cnt = np.bincount(key, minlength=NC * NB * 2).reshape(NC, NB, 2)

    T = _ceil(cnt.max(axis=0), P)  # [NB, 2] tiles per (block, half), shared
    Tb = T.sum(axis=1)             # tiles per block
    n_tiles = int(Tb.sum())
    E_pad = n_tiles * P

    # tile index where (b, h) starts. Half-major ("A-major") order: all A
    # tiles (block-major), then all B tiles — pass-level e reads and one-hot
    # source slices are then contiguous across consecutive blocks.
    tile_off = np.zeros((NB, 2), np.int64)
    run = 0
    for h in range(2):
        for b in range(NB):
            tile_off[b, h] = run
            run += T[b, h]
    assert run == n_tiles

    # gather call plan: per (block, half): calls of <= MAX_GATHER_IDX indices
    # entries: (block, half, tile_start_within_half, ntiles, idxcol_off)
    calls = []
    idxcols = 0
    for b in range(NB):
        for h in range(2):
            t0 = 0
            while t0 < T[b, h]:
                nt = min(MAX_GATHER_IDX // P, T[b, h] - t0)
                calls.append((b, h, int(t0), int(nt), idxcols))
                idxcols += nt * P // 16
                t0 += nt

    # per-core arrays
    cores = []
    counts_nodes = np.bincount(batch, minlength=G).astype(np.float32)
    for c in range(NC):
        sel = order[(dst_core[order] == c)]
        kb = blk[sel]
        kh = gsrc_half[sel]
        bucket_id = kb * 2 + kh
        o2 = np.argsort(bucket_id, kind="stable")
        sb = bucket_id[o2]
        boundaries = np.searchsorted(sb, np.arange(NB * 2))
        pos = np.zeros(len(sel), np.int64)
        pos[o2] = np.arange(len(sel)) - boundaries[sb]
        slot = (tile_off[kb, kh] * P + pos).astype(np.int64)

        eaT = np.zeros((Fe, E_pad), np.float32)
        norm_f = np.zeros(E_pad, np.float32)
        dst_f = np.zeros(E_pad, np.float32)
        gi = np.zeros(E_pad, np.int64)
        eaT[:, slot] = edge_attr[sel].T
        norm_f[slot] = norm[sel]
        dst_f[slot] = dst_in_blk[sel].astype(np.float32)
        gi[slot] = gsrc_tidx[sel]

        norm_col = np.ascontiguousarray(norm_f.reshape(n_tiles, P).T)
        dst_col = np.ascontiguousarray(dst_f.reshape(n_tiles, P).T)

        # wrapped int16 gather indices per call
        idx16 = np.zeros((P, idxcols), np.int16)
        for (b, h, ts, nt, co) in calls:
            tg = tile_off[b, h] + ts
            vals = gi[tg * P:(tg + nt) * P].astype(np.int16)
            wrapped = vals.reshape(nt * P // 16, 16).T  # [16, ni/16]
            idx16[:, co:co + nt * P // 16] = np.tile(wrapped, (8, 1))

        # nodes
        ns, ne = node_start[c], node_start[c + 1]
        nl = ne - ns
        xT = np.zeros((F, n_pad), np.float32)
        xT[:, :nl] = x[ns:ne].T
        tmp = np.zeros(n_pad, np.float32)
        tmp[:nl] = dis2[ns:ne]
        dis2_col = np.ascontiguousarray(tmp.reshape(NB, P).T)
        tmp2 = -np.ones(n_pad, np.float32)
        tmp2[:nl] = (batch[ns:ne] - c * gpc).astype(np.float32)
        batch_col = np.ascontiguousarray(tmp2.reshape(NB, P).T)
        invc = np.zeros((P, 1), np.float32)
        invc[:gpc, 0] = 1.0 / np.maximum(counts_nodes[c * gpc:(c + 1) * gpc], 1.0)

        cores.append(dict(eaT=_bf(eaT), norm_col=_bf(norm_col),
                          dst_col=_bf(dst_col), idx16=idx16, xT=_bf(xT),
                          dis2_col=_bf(dis2_col), batch_col=_bf(batch_col),
                          invc=invc))

    # full x in gather-table layout (core-major per half), replicated to all
    # cores so each builds the layer-0 table locally (t0 is linear in x)
    xTfull = np.zeros((2, F, NC * n_half), np.float32)
    for c in range(NC):
        ns, ne = node_start[c], node_start[c + 1]
        nl = ne - ns
        la = min(nl, n_half)
        xTfull[0, :, c * n_half:c * n_half + la] = x[ns:ns + la].T
        if nl > n_half:
            xTfull[1, :, c * n_half:c * n_half + (nl - n_half)] = \
                x[ns + n_half:ne].T

    plan = dict(N=N, F=F, E=E, Fe=Fe, G=G, gpc=gpc, n_pad=n_pad, NB=NB,
                NBH=NBH, n_half=n_half, T=T, Tb=Tb, n_tiles=n_tiles,
                E_pad=E_pad, tile_off=tile_off, calls=calls, idxcols=idxcols,
                cores=cores, xTfA=_bf(xTfull[0]), xTfB=_bf(xTfull[1]))
    return plan


def build_kernel(plan, weights, n_layers, sim1=False, debug=False):
    """weights: dict of numpy arrays (full, unsharded)."""
    F, Fe, NB, n_pad = plan["F"], plan["Fe"], plan["NB"], plan["n_pad"]
    NBH, n_half = plan["NBH"], plan["n_half"]
    n_tiles, E_pad = plan["n_tiles"], plan["E_pad"]
    T, tile_off, calls, idxcols = (plan["T"], plan["tile_off"], plan["calls"],
                                   plan["idxcols"])
    gpc = plan["gpc"]
    L = n_layers
    TAmax = int(T[:, 0].max())
    TBmax = int(T[:, 1].max())
    NRH = NC * n_half  # rows of each half gather table

    nc = bacc.Bacc("TRN2", target_bir_lowering=False, debug=False,
                   num_devices=(1 if sim1 else NC), num_swdge_queues=4)

    def inp(name, shape, dtype=BF16):
        return nc.dram_tensor(name, list(shape), dtype, kind="ExternalInput")

    d_eaT = inp("eaT", (Fe, E_pad))
    d_norm = inp("norm_col", (P, n_tiles))
    d_dst = inp("dst_col", (P, n_tiles))
    d_idx = inp("idx16", (P, idxcols), mybir.dt.int16)
    d_xT = inp("xT", (F, n_pad))
    d_xTfA = inp("xTfA", (F, NC * n_half))
    d_xTfB = inp("xTfB", (F, NC * n_half))
    d_W0p = inp("W0p", (F, H))
    d_b0pr = inp("b0p_rep", (1, CHUNK * H))
    d_dis2 = inp("dis2_col", (P, NB))
    d_batch = inp("batch_col", (P, NB))
    d_invc = inp("invc", (P, 1), F32)
    d_iota = inp("iota128", (P, P))
    d_iota32 = inp("iota32", (P, gpc))
    d_ident = inp("identity", (P, P), F32)
    d_identb = inp("identityb", (P, P))
    d_Wn = inp("W_node", (F, H))
    d_bn = inp("b_node", (1, H))
    d_We1 = inp("W_e1", (Fe, H))
    d_be1 = inp("b_e1", (H, 1), F32)
    d_We2 = inp("W_e2", (H, H))
    d_be2r = inp("b_e2_rep", (1, CHUNK * H))
    d_Wc = inp("W_convs", (L, H, H))
    d_bc = inp("b_convs", (L, H))
    d_Wl1 = inp("W_l1", (H, H), F32)
    d_bl1 = inp("b_l1", (1, H), F32)
    d_Wl2 = inp("W_l2", (H, 1), F32)
    b_l2_val = float(np.asarray(weights["b_l2"]).reshape(-1)[0])
    d_out = nc.dram_tensor("out", [gpc, 1], F32, kind="ExternalOutput")
    if debug:
        d_dbg_h = [nc.dram_tensor(f"dbg_h{i}", [P, NB * P], BF16,
                                  kind="ExternalOutput") for i in range(L + 1)]
        d_dbg_t = [nc.dram_tensor(f"dbg_t{i}", [P, NB * P], BF16,
                                  kind="ExternalOutput") for i in range(L)]
        d_dbg_e = nc.dram_tensor("dbg_e", [P, n_tiles * H], BF16,
                                 kind="ExternalOutput")
        d_dbg_hacc = nc.dram_tensor("dbg_hacc", [P, NB * P], F32,
                                    kind="ExternalOutput")
        d_dbg_g = nc.dram_tensor("dbg_g", [gpc, H], F32, kind="ExternalOutput")

    with tile.TileContext(nc) as tc:
        with tc.tile_pool(name="cst", bufs=1) as cst, \
             tc.tile_pool(name="big", bufs=1) as bigp, \
             tc.tile_pool(name="gw", bufs=8) as gw, \
             tc.tile_pool(name="ew", bufs=3) as ew, \
             tc.tile_pool(name="small", bufs=4) as small, \
             tc.tile_pool(name="psE", bufs=3, space="PSUM") as psE, \
             tc.tile_pool(name="psC", bufs=2, space="PSUM") as psC, \
             tc.tile_pool(name="psT", bufs=1, space="PSUM") as psT, \
             tc.tile_pool(name="psP", bufs=1, space="PSUM") as psP, \
             tc.tile_pool(name="dram", bufs=1, space="DRAM") as dram:

            nc.gpsimd.load_library(library_config.mlp)

            def load_const(tag, shape, src_ap, dtype=BF16):
                t = cst.tile(list(shape), dtype, tag=tag)
                nc.sync.dma_start(out=t[:], in_=src_ap)
                return t

            iota_t = load_const("iota", (P, P), d_iota[:, :])
            iota32_t = load_const("iota32", (P, gpc), d_iota32[:, :])
            ident_t = load_const("ident", (P, P), d_ident[:, :], F32)
            identb_t = load_const("identb", (P, P), d_identb[:, :])
            xT_t = load_const("xT", (F, n_pad), d_xT[:, :])
            W0p_t = load_const("W0p", (F, H), d_W0p[:, :])
            b0pr_t = load_const("b0pr", (1, CHUNK * H), d_b0pr[:, :])
            norm_t = load_const("norm", (P, n_tiles), d_norm[:, :])
            dst_t = load_const("dst", (P, n_tiles), d_dst[:, :])
            idx_t = load_const("idx", (P, idxcols), d_idx[:, :], mybir.dt.int16)
            dis2_t = load_const("dis2", (P, NB), d_dis2[:, :])
            batch_t = load_const("batch", (P, NB), d_batch[:, :])
            invc_t = load_const("invc", (P, 1), d_invc[:, :], F32)
            Wn_t = load_const("Wn", (F, H), d_Wn[:, :])
            bn_t = load_const("bn", (1, H), d_bn[:, :])
            We1_t = load_const("We1", (Fe, H), d_We1[:, :])
            be1_t = load_const("be1", (H, 1), d_be1[:, :], F32)
            We2_t = load_const("We2", (H, H), d_We2[:, :])
            be2r_t = load_const("be2r", (1, CHUNK * H), d_be2r[:, :])
            Wc_t = [load_const(f"Wc{l}", (H, H), d_Wc[l, :, :]) for l in range(L)]
            bc_t = [load_const(f"bc{l}", (1, H), d_bc[l:l + 1, :]) for l in range(L)]
            Wl1_t = load_const("Wl1", (H, H), d_Wl1[:, :], F32)
            bl1_t = load_const("bl1", (1, H), d_bl1[:, :], F32)
            Wl2_t = load_const("Wl2", (H, 1), d_Wl2[:, :], F32)
            ones_t = cst.tile([1, 512], F32, tag="ones")
            nc.vector.memset(ones_t[:], 1.0)
            onesb_t = cst.tile([1, 512], BF16, tag="onesb")
            nc.vector.memset(onesb_t[:], 1.0)

            h_t = bigp.tile([P, NB * P], BF16, tag="h")
            t_t = bigp.tile([P, NB * P], BF16, tag="t")
            h_acc = bigp.tile([P, NB * P], F32, tag="hacc")
            diag_all = bigp.tile([P, NB * P], BF16, tag="diag")
            ohp_all = bigp.tile([P, NB * gpc], BF16, tag="ohp")

            # prebuild per-block self-loop diagonals and pooling one-hots
            nc.vector.tensor_tensor(
                out=diag_all[:].rearrange("p (b d) -> p b d", d=P),
                in0=identb_t[:].rearrange("p (o d) -> p o d", o=1)
                    .broadcast_to([P, NB, P]),
                in1=dis2_t[:].rearrange("p (b o) -> p b o", o=1)
                    .broadcast_to([P, NB, P]),
                op=mybir.AluOpType.mult)
            nc.vector.tensor_tensor(
                out=ohp_all[:].rearrange("p (b g) -> p b g", g=gpc),
                in0=batch_t[:].rearrange("p (b o) -> p b o", o=1)
                    .broadcast_to([P, NB, gpc]),
                in1=iota32_t[:].rearrange("p (o g) -> p o g", o=1)
                    .broadcast_to([P, NB, gpc]),
                op=mybir.AluOpType.is_equal)

            e_dram = dram.tile([P, n_tiles * H], BF16, name="e_dram")
            t_locA = [dram.tile([n_half, H], BF16, name=f"t_locA{l}")
                      for l in range(L)]
            t_locB = [dram.tile([n_half, H], BF16, name=f"t_locB{l}")
                      for l in range(L)]
            # layer 0's table is computed locally (t0 linear in x) -> Local
            addr_sp = "Local" if sim1 else "Shared"
            t_fullA = [dram.tile([NRH, H], BF16,
                                 addr_space=("Local" if l == 0 else addr_sp),
                                 name=f"t_fullA{l}") for l in range(L)]
            t_fullB = [dram.tile([NRH, H], BF16,
                                 addr_space=("Local" if l == 0 else addr_sp),
                                 name=f"t_fullB{l}") for l in range(L)]

            AF = mybir.ActivationFunctionType

            def all_gather(l, half):
                assert l >= 1  # layer 0's table is built locally
                t_loc = (t_locA if half == 0 else t_locB)[l]
                t_full = (t_fullA if half == 0 else t_fullB)[l]
                if sim1:
                    for r in range(NC):
                        nc.sync.dma_start(
                            out=t_full[r * n_half:(r + 1) * n_half, :],
                            in_=t_loc[:, :])
                else:
                    nc.gpsimd.collective_compute(
                        "AllGather", mybir.AluOpType.bypass,
                        replica_groups=[list(range(NC))],
                        ins=[t_loc[:]], outs=[t_full[:]])

            def transform(l, b):
                """t_t[:, b] = (h_t[:, b])^T -> h rows; t = h @ Wc[l]; ship."""
                bc0 = b * P
                tr_ps = psT.tile([P, P], BF16, tag="Tb")
                nc.tensor.transpose(out=tr_ps[:], in_=h_t[:, bc0:bc0 + P],
                                    identity=identb_t[:])
                hT_sb = small.tile([P, P], BF16, tag="hT")
                nc.vector.tensor_copy(hT_sb[:], tr_ps[:])
                t_ps = psT.tile([P, P], F32, tag="T")
                nc.tensor.matmul(out=t_ps[:], lhsT=hT_sb[:], rhs=Wc_t[l][:],
                                 start=True, stop=True)
                nc.scalar.activation(out=t_t[:, bc0:bc0 + P], in_=t_ps[:],
                                     func=AF.Copy)
                if l == 0:
                    return  # layer-0 table built locally; nothing to ship
                if b < NBH:
                    dst_ap = t_locA[l][b * P:(b + 1) * P, :]
                else:
                    dst_ap = t_locB[l][(b - NBH) * P:(b - NBH + 1) * P, :]
                nc.sync.dma_start(out=dst_ap, in_=t_t[:, bc0:bc0 + P])

            # ---- edge MLP chunk: e[:, tiles kk..kk+cw] (edge-major, *norm) --
            def emit_mlp_chunk(kk, cw):
                w = cw * P
                ea_t = ew.tile([Fe, CHUNK * P], BF16, tag="ea")
                nc.sync.dma_start(out=ea_t[:, :w], in_=d_eaT[:, kk * P:kk * P + w])
                h1_ps = psE.tile([P, CHUNK * P], F32, tag="E")
                nc.tensor.matmul(out=h1_ps[:, :w], lhsT=We1_t[:], rhs=ea_t[:, :w],
                                 start=True, stop=True)
                h1_sb = ew.tile([P, CHUNK * P], BF16, tag="h1s")
                nc.scalar.activation(out=h1_sb[:, :w], in_=h1_ps[:, :w],
                                     func=AF.Relu, bias=be1_t[:, 0:1])
                e_ps = psE.tile([P, CHUNK * P], F32, tag="E")
                for t in range(cw):
                    # start=True zeroes the whole 2KB zero-region, so only
                    # the first matmul of the chunk may set it
                    nc.tensor.matmul(out=e_ps[:, t * H:(t + 1) * H],
                                     lhsT=h1_sb[:, t * P:(t + 1) * P],
                                     rhs=We2_t[:],
                                     start=(t == 0), stop=False)
                nc.tensor.matmul(out=e_ps[:, :w], lhsT=onesb_t[0:1, :P],
                                 rhs=be2r_t[:, :w], start=False, stop=True)
                e_sb = ew.tile([P, CHUNK * P], BF16, tag="es")
                nc.vector.tensor_tensor(
                    out=e_sb[:, :w].rearrange("p (t h) -> p t h", h=H),
                    in0=e_ps[:, :w].rearrange("p (t h) -> p t h", h=H),
                    in1=norm_t[:, kk:kk + cw]
                        .rearrange("p (t o) -> p t o", o=1)
                        .broadcast_to([P, cw, H]),
                    op=mybir.AluOpType.mult)
                nc.sync.dma_start(out=e_dram[:, kk * H:kk * H + w],
                                  in_=e_sb[:, :w])

            # ---- aggregation over one half of the gather table ----
            WAVE = 4  # blocks per e-prefetch DMA
            EWmax = 0
            for h2 in range(2):
                for w0 in range(0, NB, WAVE):
                    EWmax = max(EWmax, int(T[w0:w0 + WAVE, h2].sum()))

            def agg_pass(l, half, post_block, close=False):
                """Per wave of WAVE blocks: one e read, all gathers, one
                one-hot build, one g*e multiply; then per-block one-hot
                matmul accumulations handed to post_block(b, agg_ps)."""
                t_full = (t_fullA if half == 0 else t_fullB)[l]
                for w0 in range(0, NB, WAVE):
                    blocks = range(w0, min(w0 + WAVE, NB))
                    wt0 = int(tile_off[w0, half])
                    wtiles = int(T[w0:w0 + WAVE, half].sum())
                    if wtiles == 0:
                        for b in blocks:
                            post_block(b, None)
                        continue
                    e_t = gw.tile([P, EWmax * P], BF16, tag="ew",
                                  bufs=2)
                    nc.sync.dma_start(
                        out=e_t[:, :wtiles * P],
                        in_=e_dram[:, wt0 * H:(wt0 + wtiles) * H])
                    g_t = gw.tile([P, EWmax * P], BF16, tag="gw",
                                  bufs=2)
                    for b in blocks:
                        boff = int(tile_off[b, half]) - wt0
                        for (cb, ch, cts, cnt_, cco) in calls:
                            if cb != b or ch != half:
                                continue
                            ni = cnt_ * P
                            # queue_num is rewritten post-compile to match
                            # the scheduled-order DMASW lane assignment
                            nc.gpsimd.dma_gather(
                                out_ap=g_t[:, (boff + cts) * P:
                                           (boff + cts + cnt_) * P]
                                    .rearrange("p (j h) -> p j h", h=H),
                                in_ap=t_full[0:NRH, :],
                                idxs_ap=idx_t[:, cco:cco + ni // 16],
                                num_idxs=ni, num_idxs_reg=ni, elem_size=H,
                                queue_num=0)
                    oh_t = gw.tile([P, EWmax * P], BF16, tag="ohw",
                                   bufs=2)
                    nc.vector.tensor_tensor(
                        out=oh_t[:, :wtiles * P].rearrange("p (t d) -> p t d", d=P),
                        in0=dst_t[:, wt0:wt0 + wtiles]
                            .rearrange("p (t o) -> p t o", o=1)
                            .broadcast_to([P, wtiles, P]),
                        in1=iota_t[:].rearrange("p (o d) -> p o d", o=1)
                            .broadcast_to([P, wtiles, P]),
                        op=mybir.AluOpType.is_equal)
                    nc.vector.tensor_mul(out=g_t[:, :wtiles * P],
                                         in0=g_t[:, :wtiles * P],
                                         in1=e_t[:, :wtiles * P])
                    for b in blocks:
                        Tbh = int(T[b, half])
                        if Tbh == 0:
                            post_block(b, None)
                            continue
                        boff = int(tile_off[b, half]) - wt0
                        agg_ps = psC.tile([P, H], F32, tag="C")
                        for k in range(Tbh):
                            ko = (boff + k) * P
                            nc.tensor.matmul(out=agg_ps[:],
                                             lhsT=oh_t[:, ko:ko + P],
                                             rhs=g_t[:, ko:ko + P],
                                             start=(k == 0),
                                             stop=(close and k == Tbh - 1))
                        post_block(b, agg_ps)

            # ============ layer-0 gather table, built locally ============
            # t0 = x @ (Wn@Wc0) + bn@Wc0 is linear in the (fully available)
            # input x, so each core computes the whole table itself instead
            # of AllGathering it: kills 2 of the 6 collectives.
            for half in range(2):
                xTf = d_xTfA if half == 0 else d_xTfB
                tfull0 = (t_fullA if half == 0 else t_fullB)[0]
                for g0 in range(0, NRH // P, CHUNK):
                    cw = min(CHUNK, NRH // P - g0)
                    w = cw * P
                    xc = ew.tile([F, CHUNK * P], BF16, tag="xf")
                    nc.sync.dma_start(out=xc[:, :w],
                                      in_=xTf[:, g0 * P:g0 * P + w])
                    t_ps = psE.tile([P, CHUNK * P], F32, tag="E")
                    for t in range(cw):
                        nc.tensor.matmul(out=t_ps[:, t * H:(t + 1) * H],
                                         lhsT=xc[:, t * P:(t + 1) * P],
                                         rhs=W0p_t[:],
                                         start=(t == 0), stop=False)
                    nc.tensor.matmul(out=t_ps[:, :w], lhsT=onesb_t[0:1, :P],
                                     rhs=b0pr_t[:, :w], start=False, stop=True)
                    t_sb = ew.tile([P, CHUNK * P], BF16, tag="tf")
                    nc.scalar.activation(out=t_sb[:, :w], in_=t_ps[:, :w],
                                         func=AF.Copy)
                    nc.sync.dma_start(
                        out=tfull0[g0 * P:(g0 + cw) * P, :]
                            .rearrange("(t p) h -> p t h", p=P),
                        in_=t_sb[:, :w].rearrange("p (t h) -> p t h", h=H))

            # ============ layer 0 node embedding + transforms ============
            for b in range(NB):
                h0_ps = psC.tile([P, H], F32, tag="C")
                nc.tensor.matmul(out=h0_ps[:], lhsT=xT_t[:, b * P:(b + 1) * P],
                                 rhs=Wn_t[:], start=True, stop=False)
                nc.tensor.matmul(out=h0_ps[:], lhsT=onesb_t[0:1, :P], rhs=bn_t[:],
                                 start=False, stop=True)
                nc.scalar.activation(out=h_t[:, b * P:(b + 1) * P], in_=h0_ps[:],
                                     func=AF.Copy)
                transform(0, b)
            if debug:
                nc.sync.dma_start(out=d_dbg_h[0][:, :], in_=h_t[:])
                nc.sync.dma_start(out=d_dbg_t[0][:, :], in_=t_t[:])

            # ============ GCN layers ============
            mlp_kk = [0]

            def mlp_advance_to(tile_target):
                while mlp_kk[0] < tile_target:
                    cw = min(CHUNK, n_tiles - mlp_kk[0])
                    emit_mlp_chunk(mlp_kk[0], cw)
                    mlp_kk[0] += cw

            for l in range(L):
                # ---- pass A ----
                def flushA(b, agg_ps, l=l):
                    bc0 = b * P
                    if agg_ps is None:
                        nc.vector.memset(h_acc[:, bc0:bc0 + P], 0.0)
                        return
                    nc.scalar.activation(out=h_acc[:, bc0:bc0 + P],
                                         in_=agg_ps[:], func=AF.Copy)

                if l == 0:
                    # all MLP work first: engines drain it while the layer-0
                    # collectives run; pass A's gather-dependent ops would
                    # otherwise block later MLP ops on the in-order engines
                    mlp_advance_to(n_tiles)
                agg_pass(l, 0, flushA, close=True)

                # ---- pass B (+ fused transform/pooling + next collective) --
                def flushB(b, agg_ps, l=l):
                    bc0 = b * P
                    first = agg_ps is None
                    if first:
                        agg_ps = psC.tile([P, H], F32, tag="C")
                    nc.tensor.matmul(out=agg_ps[:],
                                     lhsT=diag_all[:, bc0:bc0 + P],
                                     rhs=t_t[:, bc0:bc0 + P],
                                     start=first, stop=False)
                    nc.tensor.matmul(out=agg_ps[:], lhsT=onesb_t[0:1, :P],
                                     rhs=bc_t[l][:], start=False, stop=True)
                    sum_sb = small.tile([P, H], F32, tag="sum")
                    nc.vector.tensor_tensor(out=sum_sb[:], in0=agg_ps[:],
                                            in1=h_acc[:, bc0:bc0 + P],
                                            op=mybir.AluOpType.add)
                    nc.scalar.activation(out=h_t[:, bc0:bc0 + P], in_=sum_sb[:],
                                         func=AF.Relu)

                pending = []

                def postB(b, agg_ps, l=l):
                    flushB(b, agg_ps)
                    # deferred by one block so PE isn't stalled on relu(b)
                    if pending:
                        pb = pending.pop()
                        emit_post_transform(l, pb)
                    pending.append(b)

                def emit_post_transform(l2, b):
                    if l2 < L - 1:
                        transform(l2 + 1, b)
                        if b == NBH - 1:
                            all_gather(l2 + 1, 0)
                        elif b == NB - 1:
                            all_gather(l2 + 1, 1)
                    else:
                        # fused global mean pool accumulation
                        nc.tensor.matmul(
                            out=g_ps[:],
                            lhsT=ohp_all[:, b * gpc:(b + 1) * gpc],
                            rhs=h_t[:, b * P:(b + 1) * P],
                            start=(b == 0), stop=(b == NB - 1))

                if l == L - 1:
                    g_ps = psP.tile([gpc, H], F32, tag="P")

                if debug and l == 0:
                    nc.sync.dma_start(out=d_dbg_e[:, :], in_=e_dram[:, :])
                    nc.sync.dma_start(out=d_dbg_hacc[:, :], in_=h_acc[:])
                agg_pass(l, 1, postB)
                if pending:
                    emit_post_transform(l, pending.pop())
                if debug:
                    nc.sync.dma_start(out=d_dbg_h[l + 1][:, :], in_=h_t[:])
                    if l < L - 1:
                        nc.sync.dma_start(out=d_dbg_t[l + 1][:, :], in_=t_t[:])

            # ---- finish pooling ----
            g_sb = small.tile([gpc, H], F32, tag="gsb")
            nc.vector.tensor_scalar(out=g_sb[:], in0=g_ps[:],
                                    scalar1=invc_t[:gpc, 0:1], scalar2=None,
                                    op0=mybir.AluOpType.mult)
            if debug:
                nc.sync.dma_start(out=d_dbg_g[:, :], in_=g_sb[:])

            # ---- head ----
            gT_ps = psT.tile([P, P], F32, tag="T")
            nc.tensor.transpose(out=gT_ps[:, :gpc], in_=g_sb[:],
                                identity=ident_t[:gpc, :gpc])
            gT_sb = small.tile([P, gpc], F32, tag="gT")
            nc.vector.tensor_copy(gT_sb[:, :], gT_ps[:, :gpc])
            z1_ps = psT.tile([P, P], F32, tag="T")
            nc.tensor.matmul(out=z1_ps[:gpc, :], lhsT=gT_sb[:], rhs=Wl1_t[:],
                             start=True, stop=False)
            nc.tensor.matmul(out=z1_ps[:gpc, :], lhsT=ones_t[0:1, :gpc],
                             rhs=bl1_t[:], start=False, stop=True)
            z1_sb = small.tile([gpc, H], F32, tag="z1")
            nc.scalar.activation(out=z1_sb[:], in_=z1_ps[:gpc, :], func=AF.Relu)
            z1T_ps = psC.tile([P, H], F32, tag="C")
            nc.tensor.transpose(out=z1T_ps[:, :gpc], in_=z1_sb[:],
                                identity=ident_t[:gpc, :gpc])
            z1T_sb = small.tile([P, gpc], F32, tag="z1T")
            nc.vector.tensor_copy(z1T_sb[:, :], z1T_ps[:, :gpc])
            o2_ps = psT.tile([P, P], F32, tag="T")
            nc.tensor.matmul(out=o2_ps[:gpc, 0:1], lhsT=z1T_sb[:], rhs=Wl2_t[:],
                             start=True, stop=True)
            out_sb = small.tile([gpc, 1], F32, tag="osb")
            nc.vector.tensor_scalar(out=out_sb[:], in0=o2_ps[:gpc, 0:1],
                                    scalar1=b_l2_val, scalar2=None,
                                    op0=mybir.AluOpType.add)
            nc.sync.dma_start(out=d_out[:, :], in_=out_sb[:])

    nc.compile()
    # Post-compile: assign gather queues round-robin in SCHEDULED order so
    # queue_num always matches the DMASW sem lane the tile scheduler binds
    # (lane = scheduled index % 8, queue = lane % num_queues).
    from concourse.tile_sem_assignment import DMAInst
    idx = 0
    for bb in nc.m.functions[0].blocks:
        for inst in bb.instructions:
            if isinstance(inst, DMAInst) and inst.engine == mybir.EngineType.Pool:
                inst.queue_num = (idx % 8) % 4
                idx += 1
    return nc


def make_in_maps(plan, weights, n_layers):
    L = n_layers
    iota128 = np.tile(np.arange(P, dtype=np.float32), (P, 1))
    iota32 = np.tile(np.arange(plan["gpc"], dtype=np.float32), (P, 1))
    ident = np.eye(P, dtype=np.float32)
    w = {k: np.asarray(v, np.float32) for k, v in weights.items()}
    be2 = w["b_e2"].reshape(1, H)
    Wc0 = w["W_convs"].reshape(L, H, H)[0]
    W0p = w["W_node"] @ Wc0
    b0p = (w["b_node"].reshape(1, H) @ Wc0)
    shared = dict(
        iota128=_bf(iota128), iota32=_bf(np.ascontiguousarray(iota32)),
        identity=ident, identityb=_bf(ident),
        xTfA=plan["xTfA"], xTfB=plan["xTfB"],
        W0p=_bf(W0p), b0p_rep=_bf(np.tile(b0p, (1, CHUNK))),
        W_node=_bf(w["W_node"]), b_node=_bf(w["b_node"].reshape(1, H)),
        W_e1=_bf(w["W_e1"]), b_e1=w["b_e1"].reshape(H, 1),
        W_e2=_bf(w["W_e2"]), b_e2_rep=_bf(np.tile(be2, (1, CHUNK))),
        W_convs=_bf(w["W_convs"].reshape(L, H, H)),
        b_convs=_bf(w["b_convs"].reshape(L, H)),
        W_l1=w["W_l1"], b_l1=w["b_l1"].reshape(1, H),
        W_l2=w["W_l2"].reshape(H, 1),
    )
    in_maps = []
    for c in range(NC):
        m = dict(shared)
        cc = plan["cores"][c]
        m.update(eaT=cc["eaT"], norm_col=cc["norm_col"], dst_col=cc["dst_col"],
                 idx16=cc["idx16"], xT=cc["xT"], dis2_col=cc["dis2_col"],
                 batch_col=cc["batch_col"], invc=cc["invc"])
        in_maps.append(m)
    return in_maps


# ----------------------------------------------------------------------------
# Public entry: kernel(**inputs) -> [256, 1] float32
# ----------------------------------------------------------------------------
N_GRAPHS = 256
N_LAYERS = 3


def _build_for_inputs(inputs):
    plan = preprocess(inputs["x"], inputs["edge_attr"], inputs["edge_index"],
                      inputs["batch"], N_GRAPHS)
    wkeys = ["W_node", "b_node", "W_e1", "b_e1", "W_e2", "b_e2", "W_convs",
             "b_convs", "W_l1", "b_l1", "W_l2", "b_l2"]
    w = {k: np.asarray(inputs[k], np.float32) for k in wkeys}
    nc_ = build_kernel(plan, w, N_LAYERS)
    in_maps = make_in_maps(plan, w, N_LAYERS)
    return nc_, in_maps, plan


def kernel(**inputs):
    from concourse.bass_utils import run_bass_kernel_spmd
    inputs = {k: np.asarray(v) for k, v in inputs.items()}
    nc_, in_maps, plan = _build_for_inputs(inputs)
    res = run_bass_kernel_spmd(nc_, in_maps, core_ids=list(range(NC)))
    out = np.concatenate([res.results[c]["out"] for c in range(NC)], axis=0)
    return out.astype(np.float32)



# revision 8
# speedup vs baseline: 1.7522x; 1.7522x over previous
"""CSGNN Trainium kernel v3: host preprocessing + Bass/Tile kernel builder.

Data-parallel over graphs: nodes partitioned at graph boundaries across 8
cores, edges live on their dst node's core grouped by 128-node dst blocks.

v3 structure (heavy tensors bf16, e in fp8):
- Layer-0 gather table t0 = x @ (Wn@Wc0) + bn@Wc0 is linear in the input,
  so it is computed ON HOST and passed as an input (t0A/t0B full tables +
  t0loc per-core slice). The device never touches x: no layer-0 embedding,
  no startup table build, gathers can start immediately.
- e (edge MLP output * norm) is built ONCE into an SBUF-resident fp8 tile
  (n_tiles*H per partition) and read by all 3 layers' aggregation waves:
  no e DRAM round trip. The MLP is interleaved with layer-0 aggregation
  waves (pre_wave callback advances the MLP just ahead of each wave).
- Gather pad slots use index -1 (HW skips trailing negative indices), so
  the ~19% tile padding costs no DMA bytes. Gathers are SWDGE-drain-bound
  (4 queues x ~22.5GB/s), so every byte off that path counts.
- Aggregation runs in two passes per layer (A-half -> bf16 accumulator,
  then B-half + self-loop via scalar_tensor_tensor + bias + relu); per
  layer t = h @ W is AllGathered in TWO chunks issued mid-pass-B so the
  collective overlaps with aggregation.
- Gather queue_nums are rewritten post-compile to match the scheduler's
  DMASW sem-lane order (emission-order rotation races after reordering).
"""
import numpy as np
import ml_dtypes

import concourse.bacc as bacc
import concourse.bass as bass
import concourse.tile as tile
import concourse.mybir as mybir
from concourse import library_config

F32 = mybir.dt.float32
BF16 = mybir.dt.bfloat16
FP8 = mybir.dt.float8e4
NC = 8
H = 128
P = 128
MAX_GATHER_IDX = 1024  # per dma_gather call (HW hangs at 2048)
CHUNK = 4  # edge-MLP tiles per chunk (512 edges)


def _ceil(a, b):
    return -(-a // b)


def _bf(x):
    return np.asarray(x).astype(ml_dtypes.bfloat16)


def preprocess(x, edge_attr, edge_index, batch, n_graphs):
    """Compute the sharding plan + per-core host arrays."""
    x = np.asarray(x, np.float32)
    edge_attr = np.asarray(edge_attr, np.float32)
    ei = np.asarray(edge_index, np.int64)
    batch = np.asarray(batch, np.int64)
    N, F = x.shape
    E, Fe = edge_attr.shape
    G = n_graphs
    assert G % NC == 0
    gpc = G // NC  # graphs per core

    src, dst = ei[0], ei[1]

    # node partition at graph boundaries
    node_start = np.searchsorted(batch, np.arange(0, G + 1, gpc), side="left")
    n_local = np.diff(node_start)
    n_pad = max(_ceil(int(n_local.max()), 2 * P) * 2 * P, 2 * P)
    NB = n_pad // P
    n_half = n_pad // 2
    NBH = NB // 2  # blocks per half
    assert NC * n_half < 32768  # int16 gather indices per chunk table

    # degrees / normalization (index-derived scalars)
    deg = 1.0 + np.bincount(dst, minlength=N).astype(np.float32)
    dis = 1.0 / np.sqrt(deg)
    norm = (dis[src] * dis[dst]).astype(np.float32)
    dis2 = (dis * dis).astype(np.float32)

    core_of = np.searchsorted(node_start[1:], np.arange(N), side="right")
    local_of = np.arange(N) - node_start[core_of]
    half_of = (local_of >= n_half).astype(np.int64)
    # row in the half-chunk gather table
    tidx = core_of * n_half + (local_of - half_of * n_half)

    gsrc_half = half_of[src]
    gsrc_tidx = tidx[src]
    dst_core = core_of[dst]
    dst_local = local_of[dst]
    blk = dst_local // P
    dst_in_blk = dst_local % P

    # bucket (core, block, half-of-src)
    key = (dst_core * NB + blk) * 2 + gsrc_half
    order = np.argsort(key, kind="stable")
    cnt = np.bincount(key, minlength=NC * NB * 2).reshape(NC, NB, 2)

    cnt_max = cnt.max(axis=0)      # [NB, 2] valid slots per bucket, shared
    T = _ceil(cnt_max, P)          # [NB, 2] tiles per (block, half), shared
    Tb = T.sum(axis=1)             # tiles per block
    n_tiles = int(Tb.sum())
    E_pad = n_tiles * P

    # tile index where (b, h) starts. Half-major ("A-major") order: all A
    # tiles (block-major), then all B tiles — pass-level one-hot source
    # slices are then contiguous across consecutive blocks.
    tile_off = np.zeros((NB, 2), np.int64)
    run = 0
    for h in range(2):
        for b in range(NB):
            tile_off[b, h] = run
            run += T[b, h]
    assert run == n_tiles

    # gather call plan: per (block, half): calls of <= MAX_GATHER_IDX indices
    # entries: (block, half, tile_start, ntiles, idxcol_off, n_valid).
    # n_valid (same across cores; slots beyond a core's own count point at
    # row 0 and are killed by e=0) feeds num_idxs_reg; the tile-rounding
    # tail uses index -1, which the gather HW skips (no DMA bytes).
    calls = []
    idxcols = 0
    for b in range(NB):
        for h in range(2):
            t0 = 0
            while t0 < T[b, h]:
                nt = min(MAX_GATHER_IDX // P, T[b, h] - t0)
                nv = int(min(max(cnt_max[b, h] - t0 * P, 0), nt * P))
                assert nv > 0
                calls.append((b, h, int(t0), int(nt), idxcols, nv))
                idxcols += nt * P // 16
                t0 += nt

    # per-core arrays
    cores = []
    counts_nodes = np.bincount(batch, minlength=G).astype(np.float32)
    for c in range(NC):
        sel = order[(dst_core[order] == c)]
        kb = blk[sel]
        kh = gsrc_half[sel]
        bucket_id = kb * 2 + kh
        o2 = np.argsort(bucket_id, kind="stable")
        sb = bucket_id[o2]
        boundaries = np.searchsorted(sb, np.arange(NB * 2))
        pos = np.zeros(len(sel), np.int64)
        pos[o2] = np.arange(len(sel)) - boundaries[sb]
        slot = (tile_off[kb, kh] * P + pos).astype(np.int64)

        eaT = np.zeros((Fe, E_pad), np.float32)
        norm_f = np.zeros(E_pad, np.float32)
        dst_f = np.zeros(E_pad, np.float32)
        # slots [cnt_core, cnt_max) gather row 0 (killed by e=0); slots
        # beyond cnt_max are -1: HW skips trailing negative gather indices
        gi = -np.ones(E_pad, np.int64)
        for b in range(NB):
            for h in range(2):
                s0 = int(tile_off[b, h]) * P
                gi[s0:s0 + int(cnt_max[b, h])] = 0
        eaT[:, slot] = edge_attr[sel].T
        norm_f[slot] = norm[sel]
        dst_f[slot] = dst_in_blk[sel].astype(np.float32)
        gi[slot] = gsrc_tidx[sel]

        norm_col = np.ascontiguousarray(norm_f.reshape(n_tiles, P).T)
        dst_col = np.ascontiguousarray(dst_f.reshape(n_tiles, P).T)

        # wrapped int16 gather indices per call
        idx16 = np.zeros((P, idxcols), np.int16)
        for (b, h, ts, nt, co, nv) in calls:
            tg = tile_off[b, h] + ts
            vals = gi[tg * P:(tg + nt) * P].astype(np.int16)
            wrapped = vals.reshape(nt * P // 16, 16).T  # [16, ni/16]
            idx16[:, co:co + nt * P // 16] = np.tile(wrapped, (8, 1))

        # nodes
        ns, ne = node_start[c], node_start[c + 1]
        nl = ne - ns
        tmp = np.zeros(n_pad, np.float32)
        tmp[:nl] = dis2[ns:ne]
        dis2_col = np.ascontiguousarray(tmp.reshape(NB, P).T)
        tmp2 = -np.ones(n_pad, np.float32)
        tmp2[:nl] = (batch[ns:ne] - c * gpc).astype(np.float32)
        batch_col = np.ascontiguousarray(tmp2.reshape(NB, P).T)
        invc = np.zeros((P, 1), np.float32)
        invc[:gpc, 0] = 1.0 / np.maximum(counts_nodes[c * gpc:(c + 1) * gpc], 1.0)

        cores.append(dict(eaT=_bf(eaT), norm_col=_bf(norm_col),
                          dst_col=_bf(dst_col), idx16=idx16,
                          dis2_col=_bf(dis2_col), batch_col=_bf(batch_col),
                          invc=invc))

    # full x in gather-table layout (core-major per half): basis for the
    # host-computed layer-0 tables (t0 is linear in x)
    xTfull = np.zeros((2, F, NC * n_half), np.float32)
    for c in range(NC):
        ns, ne = node_start[c], node_start[c + 1]
        nl = ne - ns
        la = min(nl, n_half)
        xTfull[0, :, c * n_half:c * n_half + la] = x[ns:ns + la].T
        if nl > n_half:
            xTfull[1, :, c * n_half:c * n_half + (nl - n_half)] = \
                x[ns + n_half:ne].T

    plan = dict(N=N, F=F, E=E, Fe=Fe, G=G, gpc=gpc, n_pad=n_pad, NB=NB,
                NBH=NBH, n_half=n_half, T=T, Tb=Tb, n_tiles=n_tiles,
                E_pad=E_pad, tile_off=tile_off, calls=calls, idxcols=idxcols,
                cores=cores, xTfull=xTfull, node_start=node_start)
    return plan


def host_t0_tables(plan, weights):
    """Layer-0 gather tables, computed on host: t0 = x @ (Wn@Wc0) + bn@Wc0."""
    w = {k: np.asarray(v, np.float32) for k, v in weights.items()}
    Wc0 = w["W_convs"].reshape(-1, H, H)[0]
    W0p = w["W_node"] @ Wc0                       # [F, H]
    b0p = w["b_node"].reshape(1, H) @ Wc0         # [1, H]
    xTfull = plan["xTfull"]                        # [2, F, NRH]
    t0A = xTfull[0].T @ W0p + b0p                  # [NRH, H]
    t0B = xTfull[1].T @ W0p + b0p
    # per-core local slice in [n_pad, H] node order (A rows then B rows)
    n_half = plan["n_half"]
    t0loc = []
    for c in range(NC):
        loc = np.concatenate([t0A[c * n_half:(c + 1) * n_half],
                              t0B[c * n_half:(c + 1) * n_half]], axis=0)
        t0loc.append(_bf(loc))
    return _bf(t0A), _bf(t0B), t0loc


def build_kernel(plan, weights, n_layers, sim1=False):
    """weights: dict of numpy arrays (full, unsharded)."""
    Fe, NB, n_pad = plan["Fe"], plan["NB"], plan["n_pad"]
    NBH, n_half = plan["NBH"], plan["n_half"]
    n_tiles = plan["n_tiles"]
    T, tile_off, calls, idxcols = (plan["T"], plan["tile_off"], plan["calls"],
                                   plan["idxcols"])
    gpc = plan["gpc"]
    L = n_layers
    NRH = NC * n_half  # rows of each half gather table

    nc = bacc.Bacc("TRN2", target_bir_lowering=False, debug=False,
                   num_devices=(1 if sim1 else NC), num_swdge_queues=4)

    def inp(name, shape, dtype=BF16):
        return nc.dram_tensor(name, list(shape), dtype, kind="ExternalInput")

    d_eaT = inp("eaT", (Fe, plan["E_pad"]))
    d_norm = inp("norm_col", (P, n_tiles))
    d_dst = inp("dst_col", (P, n_tiles))
    d_idx = inp("idx16", (P, idxcols), mybir.dt.int16)
    d_t0A = inp("t0A", (NRH, H))
    d_t0B = inp("t0B", (NRH, H))
    d_t0loc = inp("t0loc", (n_pad, H))
    d_dis2 = inp("dis2_col", (P, NB))
    d_batch = inp("batch_col", (P, NB))
    d_invc = inp("invc", (P, 1), F32)
    d_iota = inp("iota128", (P, P))
    d_iota32 = inp("iota32", (P, gpc))
    d_ident = inp("identity", (P, P), F32)
    d_identb = inp("identityb", (P, P))
    d_We1 = inp("W_e1", (Fe, H))
    d_be1 = inp("b_e1", (H, 1), F32)
    d_We2 = inp("W_e2", (H, H))
    d_be2r = inp("b_e2_rep", (1, CHUNK * H))
    d_Wc = inp("W_convs", (L, H, H))
    d_bc = inp("b_convs", (L, H))
    d_Wl1 = inp("W_l1", (H, H), F32)
    d_bl1 = inp("b_l1", (1, H), F32)
    d_Wl2 = inp("W_l2", (H, 1), F32)
    b_l2_val = float(np.asarray(weights["b_l2"]).reshape(-1)[0])
    d_out = nc.dram_tensor("out", [gpc, 1], F32, kind="ExternalOutput")

    with tile.TileContext(nc) as tc:
        with tc.tile_pool(name="cst", bufs=1) as cst, \
             tc.tile_pool(name="big", bufs=1) as bigp, \
             tc.tile_pool(name="gw", bufs=8) as gw, \
             tc.tile_pool(name="ew", bufs=3) as ew, \
             tc.tile_pool(name="small", bufs=4) as small, \
             tc.tile_pool(name="psE", bufs=3, space="PSUM") as psE, \
             tc.tile_pool(name="psC", bufs=2, space="PSUM") as psC, \
             tc.tile_pool(name="psT", bufs=1, space="PSUM") as psT, \
             tc.tile_pool(name="psP", bufs=1, space="PSUM") as psP, \
             tc.tile_pool(name="dram", bufs=1, space="DRAM") as dram:

            nc.gpsimd.load_library(library_config.mlp)

            def load_const(tag, shape, src_ap, dtype=BF16):
                t = cst.tile(list(shape), dtype, tag=tag)
                nc.sync.dma_start(out=t[:], in_=src_ap)
                return t

            iota_t = load_const("iota", (P, P), d_iota[:, :])
            iota32_t = load_const("iota32", (P, gpc), d_iota32[:, :])
            ident_t = load_const("ident", (P, P), d_ident[:, :], F32)
            identb_t = load_const("identb", (P, P), d_identb[:, :])
            norm_t = load_const("norm", (P, n_tiles), d_norm[:, :])
            dst_t = load_const("dst", (P, n_tiles), d_dst[:, :])
            idx_t = load_const("idx", (P, idxcols), d_idx[:, :], mybir.dt.int16)
            dis2_t = load_const("dis2", (P, NB), d_dis2[:, :])
            batch_t = load_const("batch", (P, NB), d_batch[:, :])
            invc_t = load_const("invc", (P, 1), d_invc[:, :], F32)
            We1_t = load_const("We1", (Fe, H), d_We1[:, :])
            be1_t = load_const("be1", (H, 1), d_be1[:, :], F32)
            We2_t = load_const("We2", (H, H), d_We2[:, :])
            be2r_t = load_const("be2r", (1, CHUNK * H), d_be2r[:, :])
            Wc_t = [load_const(f"Wc{l}", (H, H), d_Wc[l, :, :]) for l in range(L)]
            bc_t = [load_const(f"bc{l}", (1, H), d_bc[l:l + 1, :]) for l in range(L)]
            Wl1_t = load_const("Wl1", (H, H), d_Wl1[:, :], F32)
            bl1_t = load_const("bl1", (1, H), d_bl1[:, :], F32)
            Wl2_t = load_const("Wl2", (H, 1), d_Wl2[:, :], F32)
            ones_t = cst.tile([1, 512], F32, tag="ones")
            nc.vector.memset(ones_t[:], 1.0)
            onesb_t = cst.tile([1, 512], BF16, tag="onesb")
            nc.vector.memset(onesb_t[:], 1.0)

            h_t = bigp.tile([P, NB * P], BF16, tag="h")
            t_t = bigp.tile([P, NB * P], BF16, tag="t")
            h_acc = bigp.tile([P, NB * P], BF16, tag="hacc")
            ohp_all = bigp.tile([P, NB * gpc], BF16, tag="ohp")
            e_all = bigp.tile([P, n_tiles * H], FP8, tag="e")

            # pooling one-hots
            nc.vector.tensor_tensor(
                out=ohp_all[:].rearrange("p (b g) -> p b g", g=gpc),
                in0=batch_t[:].rearrange("p (b o) -> p b o", o=1)
                    .broadcast_to([P, NB, gpc]),
                in1=iota32_t[:].rearrange("p (o g) -> p o g", o=1)
                    .broadcast_to([P, NB, gpc]),
                op=mybir.AluOpType.is_equal)

            # layer-0 self-loop rows: DMA the host-computed local t0 slice
            nc.sync.dma_start(
                out=t_t[:].rearrange("p (t h) -> p t h", h=H),
                in_=d_t0loc[:, :].rearrange("(t p) h -> p t h", p=P))

            t_locA = [dram.tile([n_half, H], BF16, name=f"t_locA{l}")
                      for l in range(1, L)]
            t_locB = [dram.tile([n_half, H], BF16, name=f"t_locB{l}")
                      for l in range(1, L)]
            addr_sp = "Local" if sim1 else "Shared"
            t_fullA = [dram.tile([NRH, H], BF16, addr_space=addr_sp,
                                 name=f"t_fullA{l}") for l in range(1, L)]
            t_fullB = [dram.tile([NRH, H], BF16, addr_space=addr_sp,
                                 name=f"t_fullB{l}") for l in range(1, L)]

            def table_ap(l, half):
                if l == 0:
                    return (d_t0A if half == 0 else d_t0B)[0:NRH, :]
                return (t_fullA if half == 0 else t_fullB)[l - 1][0:NRH, :]

            AF = mybir.ActivationFunctionType

            def all_gather(l, half):
                assert l >= 1  # layer 0's table comes in as an input
                t_loc = (t_locA if half == 0 else t_locB)[l - 1]
                t_full = (t_fullA if half == 0 else t_fullB)[l - 1]
                if sim1:
                    for r in range(NC):
                        nc.sync.dma_start(
                            out=t_full[r * n_half:(r + 1) * n_half, :],
                            in_=t_loc[:, :])
                else:
                    nc.gpsimd.collective_compute(
                        "AllGather", mybir.AluOpType.bypass,
                        replica_groups=[list(range(NC))],
                        ins=[t_loc[:]], outs=[t_full[:]])

            def transform(l, b):
                """t_t[:, b] = (h_t[:, b])^T -> h rows; t = h @ Wc[l]; ship."""
                assert l >= 1
                bc0 = b * P
                tr_ps = psT.tile([P, P], BF16, tag="Tb")
                nc.tensor.transpose(out=tr_ps[:], in_=h_t[:, bc0:bc0 + P],
                                    identity=identb_t[:])
                hT_sb = small.tile([P, P], BF16, tag="hT")
                nc.vector.tensor_copy(hT_sb[:], tr_ps[:])
                t_ps = psT.tile([P, P], F32, tag="T")
                nc.tensor.matmul(out=t_ps[:], lhsT=hT_sb[:], rhs=Wc_t[l][:],
                                 start=True, stop=True)
                nc.scalar.activation(out=t_t[:, bc0:bc0 + P], in_=t_ps[:],
                                     func=AF.Copy)
                if b < NBH:
                    dst_ap = t_locA[l - 1][b * P:(b + 1) * P, :]
                else:
                    dst_ap = t_locB[l - 1][(b - NBH) * P:(b - NBH + 1) * P, :]
                nc.sync.dma_start(out=dst_ap, in_=t_t[:, bc0:bc0 + P])

            # ---- edge MLP chunk: e_all[:, tiles kk..kk+cw] (edge-major,
            # *norm, fp8) --------------------------------------------------
            def emit_mlp_chunk(kk, cw):
                w = cw * P
                ea_t = ew.tile([Fe, CHUNK * P], BF16, tag="ea")
                nc.sync.dma_start(out=ea_t[:, :w], in_=d_eaT[:, kk * P:kk * P + w])
                h1_ps = psE.tile([P, CHUNK * P], F32, tag="E")
                nc.tensor.matmul(out=h1_ps[:, :w], lhsT=We1_t[:], rhs=ea_t[:, :w],
                                 start=True, stop=True)
                h1_sb = ew.tile([P, CHUNK * P], BF16, tag="h1s")
                nc.scalar.activation(out=h1_sb[:, :w], in_=h1_ps[:, :w],
                                     func=AF.Relu, bias=be1_t[:, 0:1])
                e_ps = psE.tile([P, CHUNK * P], F32, tag="E")
                for t in range(cw):
                    # start=True zeroes the whole 2KB zero-region, so only
                    # the first matmul of the chunk may set it
                    nc.tensor.matmul(out=e_ps[:, t * H:(t + 1) * H],
                                     lhsT=h1_sb[:, t * P:(t + 1) * P],
                                     rhs=We2_t[:],
                                     start=(t == 0), stop=False)
                nc.tensor.matmul(out=e_ps[:, :w], lhsT=onesb_t[0:1, :P],
                                 rhs=be2r_t[:, :w], start=False, stop=True)
                nc.vector.tensor_tensor(
                    out=e_all[:, kk * H:kk * H + w].rearrange(
                        "p (t h) -> p t h", h=H),
                    in0=e_ps[:, :w].rearrange("p (t h) -> p t h", h=H),
                    in1=norm_t[:, kk:kk + cw]
                        .rearrange("p (t o) -> p t o", o=1)
                        .broadcast_to([P, cw, H]),
                    op=mybir.AluOpType.mult)

            # ---- aggregation buffers ----
            WAVE = 4  # blocks per wave
            EWmax = 0
            for h2 in range(2):
                for w0 in range(0, NB, WAVE):
                    EWmax = max(EWmax, int(T[w0:w0 + WAVE, h2].sum()))

            # zero the gather ring once: slots addressed by skipped (-1)
            # indices are never written, and NaN/Inf garbage x e=0 = NaN.
            # After this every value in the ring stays finite.
            for _ in range(3):
                gz = gw.tile([P, EWmax * P], BF16, tag="gw", bufs=3)
                nc.vector.memset(gz[:], 0.0)

            mlp_kk = [0]

            def mlp_advance_to(tile_target):
                while mlp_kk[0] < min(tile_target, n_tiles):
                    cw = min(CHUNK, n_tiles - mlp_kk[0])
                    emit_mlp_chunk(mlp_kk[0], cw)
                    mlp_kk[0] += cw

            # ---- aggregation over one half of the gather table ----
            def agg_pass(l, half, post_block, close=False, pre_wave=None):
                """Per wave of WAVE blocks: all gathers, one one-hot build,
                one g*e multiply; then per-block one-hot matmul
                accumulations handed to post_block(b, agg_ps)."""
                t_ap = table_ap(l, half)
                for w0 in range(0, NB, WAVE):
                    blocks = range(w0, min(w0 + WAVE, NB))
                    wt0 = int(tile_off[w0, half])
                    wtiles = int(T[w0:w0 + WAVE, half].sum())
                    if wtiles == 0:
                        for b in blocks:
                            post_block(b, None)
                        continue
                    if pre_wave is not None:
                        pre_wave(wt0 + wtiles)
                    g_t = gw.tile([P, EWmax * P], BF16, tag="gw",
                                  bufs=3)
                    for b in blocks:
                        boff = int(tile_off[b, half]) - wt0
                        for (cb, ch, cts, cnt_, cco, nv) in calls:
                            if cb != b or ch != half:
                                continue
                            ni = cnt_ * P
                            # queue_num is rewritten post-compile to match
                            # the scheduled-order DMASW lane assignment
                            nc.gpsimd.dma_gather(
                                out_ap=g_t[:, (boff + cts) * P:
                                           (boff + cts + cnt_) * P]
                                    .rearrange("p (j h) -> p j h", h=H),
                                in_ap=t_ap,
                                idxs_ap=idx_t[:, cco:cco + ni // 16],
                                num_idxs=ni, num_idxs_reg=nv, elem_size=H,
                                queue_num=0)
                    oh_t = gw.tile([P, EWmax * P], BF16, tag="ohw",
                                   bufs=3)
                    nc.vector.tensor_tensor(
                        out=oh_t[:, :wtiles * P].rearrange("p (t d) -> p t d", d=P),
                        in0=dst_t[:, wt0:wt0 + wtiles]
                            .rearrange("p (t o) -> p t o", o=1)
                            .broadcast_to([P, wtiles, P]),
                        in1=iota_t[:].rearrange("p (o d) -> p o d", o=1)
                            .broadcast_to([P, wtiles, P]),
                        op=mybir.AluOpType.is_equal)
                    nc.vector.tensor_tensor(
                        out=g_t[:, :wtiles * P],
                        in0=g_t[:, :wtiles * P],
                        in1=e_all[:, wt0 * H:(wt0 + wtiles) * H],
                        op=mybir.AluOpType.mult)
                    for b in blocks:
                        Tbh = int(T[b, half])
                        if Tbh == 0:
                            post_block(b, None)
                            continue
                        boff = int(tile_off[b, half]) - wt0
                        agg_ps = psC.tile([P, H], F32, tag="C")
                        for k in range(Tbh):
                            ko = (boff + k) * P
                            nc.tensor.matmul(out=agg_ps[:],
                                             lhsT=oh_t[:, ko:ko + P],
                                             rhs=g_t[:, ko:ko + P],
                                             start=(k == 0),
                                             stop=(close and k == Tbh - 1))
                        post_block(b, agg_ps)

            # ============ GCN layers ============
            for l in range(L):
                # ---- pass A ----
                def flushA(b, agg_ps, l=l):
                    bc0 = b * P
                    if agg_ps is None:
                        nc.vector.memset(h_acc[:, bc0:bc0 + P], 0.0)
                        return
                    nc.scalar.activation(out=h_acc[:, bc0:bc0 + P],
                                         in_=agg_ps[:], func=AF.Copy)

                agg_pass(l, 0, flushA, close=True,
                         pre_wave=(mlp_advance_to if l == 0 else None))

                # ---- pass B (+ fused transform/pooling + next collective) --
                def flushB(b, agg_ps, l=l):
                    bc0 = b * P
                    first = agg_ps is None
                    if first:
                        agg_ps = psC.tile([P, H], F32, tag="C")
                    nc.tensor.matmul(out=agg_ps[:], lhsT=onesb_t[0:1, :P],
                                     rhs=bc_t[l][:], start=first, stop=True)
                    # self-loop: s1 = t*dis2 + (aggB + bias)
                    s1 = small.tile([P, H], F32, tag="s1")
                    nc.vector.scalar_tensor_tensor(
                        out=s1[:], in0=t_t[:, bc0:bc0 + P],
                        scalar=dis2_t[:, b:b + 1], in1=agg_ps[:],
                        op0=mybir.AluOpType.mult, op1=mybir.AluOpType.add)
                    sum_sb = small.tile([P, H], F32, tag="sum")
                    nc.vector.tensor_tensor(out=sum_sb[:], in0=s1[:],
                                            in1=h_acc[:, bc0:bc0 + P],
                                            op=mybir.AluOpType.add)
                    nc.scalar.activation(out=h_t[:, bc0:bc0 + P], in_=sum_sb[:],
                                         func=AF.Relu)

                pending = []

                def postB(b, agg_ps, l=l):
                    flushB(b, agg_ps)
                    # deferred by one block so PE isn't stalled on relu(b)
                    if pending:
                        pb = pending.pop()
                        emit_post_transform(l, pb)
                    pending.append(b)

                def emit_post_transform(l2, b):
                    if l2 < L - 1:
                        transform(l2 + 1, b)
                        if b == NBH - 1:
                            all_gather(l2 + 1, 0)
                        elif b == NB - 1:
                            all_gather(l2 + 1, 1)
                    else:
                        # fused global mean pool accumulation
                        nc.tensor.matmul(
                            out=g_ps[:],
                            lhsT=ohp_all[:, b * gpc:(b + 1) * gpc],
                            rhs=h_t[:, b * P:(b + 1) * P],
                            start=(b == 0), stop=(b == NB - 1))

                if l == L - 1:
                    g_ps = psP.tile([gpc, H], F32, tag="P")

                agg_pass(l, 1, postB,
                         pre_wave=(mlp_advance_to if l == 0 else None))
                if pending:
                    emit_post_transform(l, pending.pop())

            # ---- finish pooling ----
            g_sb = small.tile([gpc, H], F32, tag="gsb")
            nc.vector.tensor_scalar(out=g_sb[:], in0=g_ps[:],
                                    scalar1=invc_t[:gpc, 0:1], scalar2=None,
                                    op0=mybir.AluOpType.mult)

            # ---- head ----
            gT_ps = psT.tile([P, P], F32, tag="T")
            nc.tensor.transpose(out=gT_ps[:, :gpc], in_=g_sb[:],
                                identity=ident_t[:gpc, :gpc])
            gT_sb = small.tile([P, gpc], F32, tag="gT")
            nc.vector.tensor_copy(gT_sb[:, :], gT_ps[:, :gpc])
            z1_ps = psT.tile([P, P], F32, tag="T")
            nc.tensor.matmul(out=z1_ps[:gpc, :], lhsT=gT_sb[:], rhs=Wl1_t[:],
                             start=True, stop=False)
            nc.tensor.matmul(out=z1_ps[:gpc, :], lhsT=ones_t[0:1, :gpc],
                             rhs=bl1_t[:], start=False, stop=True)
            z1_sb = small.tile([gpc, H], F32, tag="z1")
            nc.scalar.activation(out=z1_sb[:], in_=z1_ps[:gpc, :], func=AF.Relu)
            z1T_ps = psC.tile([P, H], F32, tag="C")
            nc.tensor.transpose(out=z1T_ps[:, :gpc], in_=z1_sb[:],
                                identity=ident_t[:gpc, :gpc])
            z1T_sb = small.tile([P, gpc], F32, tag="z1T")
            nc.vector.tensor_copy(z1T_sb[:, :], z1T_ps[:, :gpc])
            o2_ps = psT.tile([P, P], F32, tag="T")
            nc.tensor.matmul(out=o2_ps[:gpc, 0:1], lhsT=z1T_sb[:], rhs=Wl2_t[:],
                             start=True, stop=True)
            out_sb = small.tile([gpc, 1], F32, tag="osb")
            nc.vector.tensor_scalar(out=out_sb[:], in0=o2_ps[:gpc, 0:1],
                                    scalar1=b_l2_val, scalar2=None,
                                    op0=mybir.AluOpType.add)
            nc.sync.dma_start(out=d_out[:, :], in_=out_sb[:])

    nc.compile()
    # Post-compile: assign gather queues round-robin in SCHEDULED order so
    # queue_num always matches the DMASW sem lane the tile scheduler binds
    # (lane = scheduled index % 8, queue = lane % num_queues).
    from concourse.tile_sem_assignment import DMAInst
    idx = 0
    for bb in nc.m.functions[0].blocks:
        for inst in bb.instructions:
            if isinstance(inst, DMAInst) and inst.engine == mybir.EngineType.Pool:
                inst.queue_num = (idx % 8) % 4
                idx += 1
    return nc


def make_in_maps(plan, weights, n_layers):
    L = n_layers
    iota128 = np.tile(np.arange(P, dtype=np.float32), (P, 1))
    iota32 = np.tile(np.arange(plan["gpc"], dtype=np.float32), (P, 1))
    ident = np.eye(P, dtype=np.float32)
    w = {k: np.asarray(v, np.float32) for k, v in weights.items()}
    be2 = w["b_e2"].reshape(1, H)
    t0A, t0B, t0loc = host_t0_tables(plan, weights)
    shared = dict(
        iota128=_bf(iota128), iota32=_bf(np.ascontiguousarray(iota32)),
        identity=ident, identityb=_bf(ident),
        t0A=t0A, t0B=t0B,
        W_e1=_bf(w["W_e1"]), b_e1=w["b_e1"].reshape(H, 1),
        W_e2=_bf(w["W_e2"]), b_e2_rep=_bf(np.tile(be2, (1, CHUNK))),
        W_convs=_bf(w["W_convs"].reshape(L, H, H)),
        b_convs=_bf(w["b_convs"].reshape(L, H)),
        W_l1=w["W_l1"], b_l1=w["b_l1"].reshape(1, H),
        W_l2=w["W_l2"].reshape(H, 1),
    )
    in_maps = []
    for c in range(NC):
        m = dict(shared)
        cc = plan["cores"][c]
        m.update(eaT=cc["eaT"], norm_col=cc["norm_col"], dst_col=cc["dst_col"],
                 idx16=cc["idx16"], dis2_col=cc["dis2_col"],
                 batch_col=cc["batch_col"], invc=cc["invc"],
                 t0loc=t0loc[c])
        in_maps.append(m)
    return in_maps


# ----------------------------------------------------------------------------
# Public entry: kernel(**inputs) -> [256, 1] float32
# ----------------------------------------------------------------------------
N_GRAPHS = 256
N_LAYERS = 3


def _build_for_inputs(inputs):
    plan = preprocess(inputs["x"], inputs["edge_attr"], inputs["edge_index"],
                      inputs["batch"], N_GRAPHS)
    wkeys = ["W_node", "b_node", "W_e1", "b_e1", "W_e2", "b_e2", "W_convs",
             "b_convs", "W_l1", "b_l1", "W_l2", "b_l2"]
    w = {k: np.asarray(inputs[k], np.float32) for k in wkeys}
    nc_ = build_kernel(plan, w, N_LAYERS)
    in_maps = make_in_maps(plan, w, N_LAYERS)
    return nc_, in_maps, plan


def kernel(**inputs):
    from concourse.bass_utils import run_bass_kernel_spmd
    inputs = {k: np.asarray(v) for k, v in inputs.items()}
    nc_, in_maps, plan = _build_for_inputs(inputs)
    res = run_bass_kernel_spmd(nc_, in_maps, core_ids=list(range(NC)))
    out = np.concatenate([res.results[c]["out"] for c in range(NC)], axis=0)
    return out.astype(np.float32)
